# revision 10
# baseline (speedup 1.0000x reference)
"""CTC loss (keras ctc_batch_cost semantics, full-width lengths, blank=C-1)
as a Bass/Tile kernel on 8 TRN2 NeuronCores. Pure data parallel over batch.

Per core (128 batch rows on SBUF partitions):
  - stream y_pred shard, ACT permuted-downcast to bf16 [t,(c,b)] panels,
    xbar-transpose (8 batch rows per transpose) to c-major [c,(b,t)]
  - PE one-hot gather (entries = K) -> per-label probability slabs Q_k[b,t]
  - B-form s-sweep over the extended CTC lattice: each lattice row's whole
    time trajectory is one tensor_tensor_scan (state = coef[t]*state + inflow),
    all slab streams bf16 (DVE 2x-capable layout), scan state fp32
  - K = 78.5 per-step normalizer keeps trajectories inside fp32/bf16 range
  - loss = T*log(K) - log(2^64*(Bfin[T]*Rs[T-1] + Bodd[T]*Qs_{L-1}[T-1])) + 64*ln2
"""

import math
import sys
from contextlib import ExitStack

import numpy as np

sys.path.insert(0, "/opt/trn_rl_repo")

import ml_dtypes  # noqa: E402

import concourse.bass as bass  # noqa: E402
import concourse.tile as tile  # noqa: E402
from concourse import mybir  # noqa: E402
from concourse._compat import with_exitstack  # noqa: E402
from concourse.bass_utils import run_bass_kernel_spmd  # noqa: E402

# problem constants (hardcoded; harness shapes are fixed)
B_FULL = 1024
T = 512
C = 128
L = 64
NCORES = 8
BQ = B_FULL // NCORES   # 128 batch rows per core
S_PAD = T + 2           # slab block: col0 = pad (t=-1), cols 1..T = t, col T+1 dead
KVAL = 78.5             # bf16-exact per-step normalizer (calibrated offline)
EPS = 1e-7
EPSK = EPS * KVAL
LNSHIFT = 64.0 * math.log(2.0)   # Ln(z*2^64) keeps ACT Ln in its accurate domain
TLOGK = float(T * math.log(KVAL)) + LNSHIFT

FP32 = mybir.dt.float32
BF16 = mybir.dt.bfloat16
AF = mybir.ActivationFunctionType
ALU = mybir.AluOpType

TQ = 256                # t-panel width for the gather pipeline
NPANEL = T // TQ
BG = 8                  # batch elems per staging tile / batched transpose


@with_exitstack
def _ctc_tile_kernel(ctx: ExitStack, tc: tile.TileContext, outs, ins):
    nc = tc.nc
    yp, hmat, mask = ins
    (loss_out,) = outs

    consts = ctx.enter_context(tc.tile_pool(name="consts", bufs=1))
    bfpool = ctx.enter_context(tc.tile_pool(name="bfc", bufs=4))
    ytpool = ctx.enter_context(tc.tile_pool(name="yt", bufs=1))
    qpool = ctx.enter_context(tc.tile_pool(name="qs", bufs=1))
    psum = ctx.enter_context(tc.tile_pool(name="ps", bufs=4, space="PSUM"))
    stpool = ctx.enter_context(tc.tile_pool(name="stg", bufs=4))
    bpool = ctx.enter_context(tc.tile_pool(name="bslab", bufs=1))
    wpool = ctx.enter_context(tc.tile_pool(name="wrk", bufs=3))
    fpool = ctx.enter_context(tc.tile_pool(name="fin", bufs=1))

    hm = consts.tile([C, BQ * L], BF16, tag="hm")        # one-hot*K, [c, (b,k)]
    nc.sync.dma_start(hm[:], hmat[:, :])
    msk = consts.tile([BQ, L], FP32, tag="msk")
    nc.sync.dma_start(msk[:], mask[:, :])
    qs = qpool.tile([BQ, L * S_PAD], BF16, tag="qs")     # Q slabs, per-k blocks
    rs = consts.tile([BQ, S_PAD], BF16, tag="rs")        # blank slab
    qs3 = qs[:].rearrange("b (k s) -> b k s", k=L)
    nc.vector.memset(qs3[:, :, 0:1], 0.0)
    nc.vector.memset(rs[:, 0:1], 1.0)

    yt = ytpool.tile([C, BQ * TQ], BF16, tag="yt")       # c-major Y panel [c,(b,t)]

    cbias = consts.tile([BQ, 2], FP32, tag="cbias")
    nc.vector.memset(cbias[:, 0:1], EPSK)
    nc.vector.memset(cbias[:, 1:2], TLOGK)
    epsk_ap = cbias[:, 0:1]
    tlogk_ap = cbias[:, 1:2]

    # ---- stage A: SWDGE cast-stream + transpose + PE gather ----
    # t-panels of 256 for the gather; transposes at 128-t grain
    for p in range(NPANEL):
        t0 = p * TQ
        for bg in range(BQ // BG):
            for th in range(TQ // 128):
                tt = t0 + th * 128
                # casting DMA (SWDGE): DRAM fp32 [8b,128t,128c] -> SBUF bf16 [t,(b,c)]
                bfm = bfpool.tile([128, BG * C], BF16, tag="bfc")
                src_ = yp[bg * BG:(bg + 1) * BG, tt:tt + 128, :]
                nc.gpsimd.dma_start(bfm[:].rearrange("t (b c) -> t b c", b=BG),
                                    src_.rearrange("b t c -> t b c"))
                # batched transpose: xpose row index = dim1*128 + partition
                dst = yt[:, bg * BG * TQ + th * 128:]
                dst = yt[:].rearrange("c (b t) -> c b t", b=BQ)[
                    :, bg * BG:(bg + 1) * BG, th * 128:th * 128 + 128]
                nc.sync.dma_start_transpose(dst, bfm[:])
        # blank slab: yt partition C-1 holds r[b, t] for this panel
        rraw = stpool.tile([BQ, TQ], BF16, tag="rraw")
        nc.sync.dma_start(rraw[:], yt[C - 1:C, :])
        nc.scalar.activation(rs[:, 1 + t0: 1 + t0 + TQ], rraw[:], AF.Identity,
                             bias=epsk_ap, scale=KVAL)
        # PE gather per b-pair -> PSUM [128(2b), TQ], one rearrange DMA per pair
        for bp in range(BQ // 2):
            b0, b1 = 2 * bp, 2 * bp + 1
            pt = psum.tile([128, TQ], FP32, tag="ps")
            nc.tensor.matmul(pt[0:64, :], hm[:, b0 * L:(b0 + 1) * L],
                             yt[:, b0 * TQ:(b0 + 1) * TQ], start=True, stop=True)
            nc.tensor.matmul(pt[64:128, :], hm[:, b1 * L:(b1 + 1) * L],
                             yt[:, b1 * TQ:(b1 + 1) * TQ], start=True, stop=True,
                             tile_position=(0, 64))
            st = stpool.tile([128, TQ], BF16, tag="stg")
            nc.scalar.activation(st[:], pt[:], AF.Identity, bias=epsk_ap, scale=1.0)
            dst = qs[b0:b0 + 2, :].rearrange("b (k s) -> b k s", k=L)
            nc.sync.dma_start(dst[:, :, 1 + t0:1 + t0 + TQ], st[:, :])

    # ---- stage B: the s-sweep (all-bf16 streams, fp32 scan state) ----
    rsh = rs[:, 0:T]              # shifted (t-1) view, 4B-aligned
    rtop = rs[:, T:T + 1]

    def qsh(k):
        return qs[:, k * S_PAD: k * S_PAD + T]

    def qtop(k):
        return qs[:, k * S_PAD + T: k * S_PAD + T + 1]

    # B-slab ring: pads preset once
    b0s = bpool.tile([BQ, S_PAD], BF16, tag="b0")
    ring = []
    for i in range(3):
        rtile = bpool.tile([BQ, S_PAD], BF16, tag=f"br{i}")
        ring.append(rtile)
    nc.vector.memset(b0s[:, 0:1], 1.0)
    for r_ in ring:
        nc.vector.memset(r_[:, 0:1], 0.0)

    # row 0: B0 = TTS(Rsh, e0), e0 = [1,0,...]
    w = wpool.tile([BQ, T], BF16, tag="wrk")
    nc.vector.memset(w[:], 0.0)
    nc.vector.memset(w[:, 0:1], 1.0)
    b_even = b0s
    nc.vector.tensor_tensor_scan(b_even[:, 1:T + 1], rsh, w[:], 0.0,
                                 ALU.mult, ALU.add)

    # row 1 (k=0): u = Rsh*B0sh (u[0]=1); B1 = TTS(Q0sh, u)
    u = wpool.tile([BQ, T], BF16, tag="wrk")
    nc.vector.tensor_mul(u[:], rsh, b_even[:, 0:T])
    nc.vector.memset(u[:, 0:1], 1.0)
    b_odd = ring[0]
    nc.vector.tensor_tensor_scan(b_odd[:, 1:T + 1], qsh(0), u[:], 0.0,
                                 ALU.mult, ALU.add)

    for k in range(1, L):
        tmp = wpool.tile([BQ, T], BF16, tag="wrk")
        nc.vector.tensor_mul(tmp[:], qsh(k - 1), b_odd[:, 0:T])
        b_even = ring[(2 * k) % 3]
        nc.vector.tensor_tensor_scan(b_even[:, 1:T + 1], rsh, tmp[:], 0.0,
                                     ALU.mult, ALU.add)
        u = wpool.tile([BQ, T], BF16, tag="wrk")
        nc.vector.tensor_mul(u[:], rsh, b_even[:, 0:T])
        w = wpool.tile([BQ, T], BF16, tag="wrk")
        nc.vector.scalar_tensor_tensor(w[:], tmp[:], msk[:, k:k + 1], u[:],
                                       ALU.mult, ALU.add)
        b_odd = ring[(2 * k + 1) % 3]
        nc.vector.tensor_tensor_scan(b_odd[:, 1:T + 1], qsh(k), w[:], 0.0,
                                     ALU.mult, ALU.add)

    tmp = wpool.tile([BQ, T], BF16, tag="wrk")
    nc.vector.tensor_mul(tmp[:], qsh(L - 1), b_odd[:, 0:T])
    b_fin = ring[(2 * L) % 3]
    nc.vector.tensor_tensor_scan(b_fin[:, 1:T + 1], rsh, tmp[:], 0.0,
                                 ALU.mult, ALU.add)

    # ---- final extraction ----
    z1 = fpool.tile([BQ, 4], FP32, tag="fin")
    nc.vector.tensor_mul(z1[:, 0:1], b_fin[:, T:T + 1], rtop)
    nc.vector.tensor_mul(z1[:, 1:2], b_odd[:, T:T + 1], qtop(L - 1))
    nc.vector.tensor_add(z1[:, 2:3], z1[:, 0:1], z1[:, 1:2])
    nc.scalar.activation(z1[:, 3:4], z1[:, 2:3], AF.Ln, scale=float(2.0 ** 64))
    lf = fpool.tile([BQ, 1], FP32, tag="lout")
    nc.scalar.activation(lf[:], z1[:, 3:4], AF.Identity, bias=tlogk_ap, scale=-1.0)
    nc.sync.dma_start(loss_out[:, :], lf[:])


_CACHE = {}

# The walrus build in this container accepts at most ONE sem-wait condition
# per instruction. Tile emits merged multi-waits; split the extras onto
# injected standalone EventSemaphore wait instructions on the same engine.
_WAIT_TMPL = {"debug": 0, "engine": "DVE", "ins": [], "name": "W-0",
              "opcode": "EventSemaphore", "outs": [],
              "sync_info": {"on_update": [], "on_wait": []}}


def _split_multiwaits(js: bytes) -> bytes:
    import copy
    import json
    m = json.loads(js)
    ctr = 0
    for f in m["functions"]:
        for bb in f["blocks"]:
            if "instructions" not in bb:
                continue
            out = []
            for ins in bb["instructions"]:
                si = ins.get("sync_info")
                ow = (si or {}).get("on_wait") or []
                if len(ow) > 1:
                    for wcond in ow[:-1]:
                        nop = copy.deepcopy(_WAIT_TMPL)
                        nop["engine"] = ins["engine"]
                        nop["name"] = f"W-{ctr}"
                        ctr += 1
                        nop["sync_info"]["on_wait"] = [wcond]
                        out.append(nop)
                    si["on_wait"] = [ow[-1]]
                out.append(ins)
            bb["instructions"] = out
    return json.dumps(m).encode()


def _build_nc():
    if "nc" in _CACHE:
        return _CACHE["nc"]
    nc = bass.Bass("TRN2", target_bir_lowering=False, debug=False)
    yp = nc.dram_tensor("yp", [BQ, T, C], FP32, kind="ExternalInput").ap()
    hmat = nc.dram_tensor("hmat", [C, BQ * L], BF16, kind="ExternalInput").ap()
    mask = nc.dram_tensor("mask", [BQ, L], FP32, kind="ExternalInput").ap()
    loss = nc.dram_tensor("loss", [BQ, 1], FP32, kind="ExternalOutput").ap()
    with tile.TileContext(nc) as tc:
        _ctc_tile_kernel(tc, [loss], [yp, hmat, mask])
    orig = type(nc).to_json_bytes
    nc.to_json_bytes = lambda: _split_multiwaits(orig(nc))
    _CACHE["nc"] = nc
    return nc


def _host_prep(yt_shard):
    H = np.zeros((C, BQ * L), np.float32)
    cols = np.arange(BQ * L)
    H[yt_shard.reshape(-1).astype(np.int64), cols] = KVAL
    m = np.zeros((BQ, L), np.float32)
    m[:, 1:] = (yt_shard[:, 1:] != yt_shard[:, :-1]).astype(np.float32)
    return np.ascontiguousarray(H.astype(ml_dtypes.bfloat16)), m


def _run(y_true, y_pred, trace=False):
    nc = _build_nc()
    yt_np = np.asarray(y_true)
    yp_np = np.asarray(y_pred, dtype=np.float32)
    in_maps = []
    for ci in range(NCORES):
        sl = slice(ci * BQ, (ci + 1) * BQ)
        H, m = _host_prep(yt_np[sl])
        in_maps.append({
            "yp": np.ascontiguousarray(yp_np[sl]),
            "hmat": H,
            "mask": m,
        })
    res = run_bass_kernel_spmd(nc, in_maps, core_ids=list(range(NCORES)),
                               trace=trace)
    loss = np.concatenate([res.results[ci]["loss"] for ci in range(NCORES)],
                          axis=0).astype(np.float32)
    return loss, res


def kernel(y_true, y_pred):
    loss, _ = _run(y_true, y_pred, trace=False)
    return loss


# revision 14
# speedup vs baseline: 1.2991x; 1.2991x over previous
"""CTC loss (keras ctc_batch_cost semantics, full-width lengths, blank=C-1)
as a Bass/Tile kernel on 8 TRN2 NeuronCores. Pure data parallel over batch.

Per core (128 batch rows on SBUF partitions):
  - stream y_pred shard, ACT permuted-downcast to bf16 [t,(c,b)] panels,
    xbar-transpose (8 batch rows per transpose) to c-major [c,(b,t)]
  - PE one-hot gather (entries = K) -> per-label probability slabs Q_k[b,t]
  - B-form s-sweep over the extended CTC lattice: each lattice row's whole
    time trajectory is one tensor_tensor_scan (state = coef[t]*state + inflow),
    all slab streams bf16 (DVE 2x-capable layout), scan state fp32
  - K = 78.5 per-step normalizer keeps trajectories inside fp32/bf16 range
  - loss = T*log(K) - log(2^64*(Bfin[T]*Rs[T-1] + Bodd[T]*Qs_{L-1}[T-1])) + 64*ln2
"""

import math
import sys
from contextlib import ExitStack

import numpy as np

sys.path.insert(0, "/opt/trn_rl_repo")

import ml_dtypes  # noqa: E402

import concourse.bass as bass  # noqa: E402
import concourse.tile as tile  # noqa: E402
from concourse import mybir  # noqa: E402
from concourse._compat import with_exitstack  # noqa: E402
from concourse.bass_utils import run_bass_kernel_spmd  # noqa: E402

# problem constants (hardcoded; harness shapes are fixed)
B_FULL = 1024
T = 512
C = 128
L = 64
NCORES = 8
BQ = B_FULL // NCORES   # 128 batch rows per core
S_PAD = T + 2           # slab block: col0 = pad (t=-1), cols 1..T = t, col T+1 dead
KVAL = 78.5             # bf16-exact per-step normalizer (calibrated offline)
EPS = 1e-7
EPSK = EPS * KVAL
LNSHIFT = 64.0 * math.log(2.0)   # Ln(z*2^64) keeps ACT Ln in its accurate domain
TLOGK = float(T * math.log(KVAL)) + LNSHIFT

FP32 = mybir.dt.float32
BF16 = mybir.dt.bfloat16
AF = mybir.ActivationFunctionType
ALU = mybir.AluOpType

TQ = 256                # t-panel width for the gather pipeline
NPANEL = T // TQ
BG = 16                 # batch elems per staging tile / batched transpose


@with_exitstack
def _ctc_tile_kernel(ctx: ExitStack, tc: tile.TileContext, outs, ins):
    nc = tc.nc
    yp, hmat, mask = ins
    (loss_out,) = outs

    consts = ctx.enter_context(tc.tile_pool(name="consts", bufs=1))
    bfpool = ctx.enter_context(tc.tile_pool(name="bfc", bufs=1))
    ytpool = ctx.enter_context(tc.tile_pool(name="yt", bufs=1))
    qpool = ctx.enter_context(tc.tile_pool(name="qs", bufs=1))
    psum = ctx.enter_context(tc.tile_pool(name="ps", bufs=4, space="PSUM"))
    stpool = ctx.enter_context(tc.tile_pool(name="stg", bufs=6))
    bpool = ctx.enter_context(tc.tile_pool(name="bslab", bufs=1))
    wpool = ctx.enter_context(tc.tile_pool(name="wrk", bufs=3))
    fpool = ctx.enter_context(tc.tile_pool(name="fin", bufs=1))

    hm = consts.tile([C, BQ * L], BF16, tag="hm")        # one-hot*K, [c, (b,k)]
    nc.sync.dma_start(hm[:], hmat[:, :])
    msk = consts.tile([BQ, L], FP32, tag="msk")
    nc.sync.dma_start(msk[:], mask[:, :])
    qs = qpool.tile([BQ, L * S_PAD], BF16, tag="qs")     # Q slabs, per-k blocks
    rs = consts.tile([BQ, S_PAD], BF16, tag="rs")        # blank slab
    qs3 = qs[:].rearrange("b (k s) -> b k s", k=L)
    nc.vector.memset(qs3[:, :, 0:1], 0.0)
    nc.vector.memset(rs[:, 0:1], 1.0)

    yt = ytpool.tile([C, BQ * TQ], BF16, tag="yt")       # c-major Y panel [c,(b,t)]

    cbias = consts.tile([BQ, 2], FP32, tag="cbias")
    nc.vector.memset(cbias[:, 0:1], EPSK)
    nc.vector.memset(cbias[:, 1:2], TLOGK)
    epsk_ap = cbias[:, 0:1]
    tlogk_ap = cbias[:, 1:2]

    # ---- stage A: SWDGE cast-stream + transpose + PE gather ----
    # one big casting load per 128-t chunk, then a clustered transpose burst
    # (minimizes xbar-mode transitions, which serialize the DMA path)
    for p in range(NPANEL):
        t0 = p * TQ
        for th in range(TQ // 128):
            tt = t0 + th * 128
            bfall = bfpool.tile([128, BQ * C], BF16, tag="bfall")
            for hb in range(2):
                src_ = yp[hb * 64:(hb + 1) * 64, tt:tt + 128, :]
                dst_ = bfall[:, hb * 64 * C:(hb + 1) * 64 * C]
                nc.gpsimd.dma_start(dst_.rearrange("t (b c) -> t b c", b=64),
                                    src_.rearrange("b t c -> t b c"))
            for bg in range(BQ // BG):
                dst = yt[:].rearrange("c (b t) -> c b t", b=BQ)[
                    :, bg * BG:(bg + 1) * BG, th * 128:th * 128 + 128]
                nc.sync.dma_start_transpose(
                    dst, bfall[:, bg * BG * C:(bg + 1) * BG * C])
        # blank slab: yt partition C-1 holds r[b, t] for this panel
        rraw = stpool.tile([BQ, TQ], BF16, tag="rraw")
        nc.sync.dma_start(rraw[:], yt[C - 1:C, :])
        nc.scalar.activation(rs[:, 1 + t0: 1 + t0 + TQ], rraw[:], AF.Identity,
                             bias=epsk_ap, scale=KVAL)
        # PE gather per b-pair -> PSUM [128(2b), TQ], one rearrange DMA per pair
        for bp in range(BQ // 2):
            b0, b1 = 2 * bp, 2 * bp + 1
            pt = psum.tile([128, TQ], FP32, tag="ps")
            nc.tensor.matmul(pt[0:64, :], hm[:, b0 * L:(b0 + 1) * L],
                             yt[:, b0 * TQ:(b0 + 1) * TQ], start=True, stop=True)
            nc.tensor.matmul(pt[64:128, :], hm[:, b1 * L:(b1 + 1) * L],
                             yt[:, b1 * TQ:(b1 + 1) * TQ], start=True, stop=True,
                             tile_position=(0, 64))
            st = stpool.tile([128, TQ], BF16, tag="stg")
            nc.scalar.activation(st[:], pt[:], AF.Identity, bias=epsk_ap, scale=1.0)
            dst = qs[b0:b0 + 2, :].rearrange("b (k s) -> b k s", k=L)
            nc.sync.dma_start(dst[:, :, 1 + t0:1 + t0 + TQ], st[:, :])

    # ---- stage B: the s-sweep (all-bf16 streams, fp32 scan state) ----
    rsh = rs[:, 0:T]              # shifted (t-1) view, 4B-aligned
    rtop = rs[:, T:T + 1]

    def qsh(k):
        return qs[:, k * S_PAD: k * S_PAD + T]

    def qtop(k):
        return qs[:, k * S_PAD + T: k * S_PAD + T + 1]

    # B-slab ring: pads preset once
    b0s = bpool.tile([BQ, S_PAD], BF16, tag="b0")
    ring = []
    for i in range(3):
        rtile = bpool.tile([BQ, S_PAD], BF16, tag=f"br{i}")
        ring.append(rtile)
    nc.vector.memset(b0s[:, 0:1], 1.0)
    for r_ in ring:
        nc.vector.memset(r_[:, 0:1], 0.0)

    # row 0: B0 = TTS(Rsh, e0), e0 = [1,0,...]
    w = wpool.tile([BQ, T], BF16, tag="wrk")
    nc.vector.memset(w[:], 0.0)
    nc.vector.memset(w[:, 0:1], 1.0)
    b_even = b0s
    nc.vector.tensor_tensor_scan(b_even[:, 1:T + 1], rsh, w[:], 0.0,
                                 ALU.mult, ALU.add)

    # row 1 (k=0): u = Rsh*B0sh (u[0]=1); B1 = TTS(Q0sh, u)
    u = wpool.tile([BQ, T], BF16, tag="wrk")
    nc.vector.tensor_mul(u[:], rsh, b_even[:, 0:T])
    nc.vector.memset(u[:, 0:1], 1.0)
    b_odd = ring[0]
    nc.vector.tensor_tensor_scan(b_odd[:, 1:T + 1], qsh(0), u[:], 0.0,
                                 ALU.mult, ALU.add)

    for k in range(1, L):
        tmp = wpool.tile([BQ, T], BF16, tag="wrk")
        nc.vector.tensor_mul(tmp[:], qsh(k - 1), b_odd[:, 0:T])
        b_even = ring[(2 * k) % 3]
        nc.vector.tensor_tensor_scan(b_even[:, 1:T + 1], rsh, tmp[:], 0.0,
                                     ALU.mult, ALU.add)
        u = wpool.tile([BQ, T], BF16, tag="wrk")
        nc.vector.tensor_mul(u[:], rsh, b_even[:, 0:T])
        w = wpool.tile([BQ, T], BF16, tag="wrk")
        nc.vector.scalar_tensor_tensor(w[:], tmp[:], msk[:, k:k + 1], u[:],
                                       ALU.mult, ALU.add)
        b_odd = ring[(2 * k + 1) % 3]
        nc.vector.tensor_tensor_scan(b_odd[:, 1:T + 1], qsh(k), w[:], 0.0,
                                     ALU.mult, ALU.add)

    tmp = wpool.tile([BQ, T], BF16, tag="wrk")
    nc.vector.tensor_mul(tmp[:], qsh(L - 1), b_odd[:, 0:T])
    b_fin = ring[(2 * L) % 3]
    nc.vector.tensor_tensor_scan(b_fin[:, 1:T + 1], rsh, tmp[:], 0.0,
                                 ALU.mult, ALU.add)

    # ---- final extraction ----
    z1 = fpool.tile([BQ, 4], FP32, tag="fin")
    nc.vector.tensor_mul(z1[:, 0:1], b_fin[:, T:T + 1], rtop)
    nc.vector.tensor_mul(z1[:, 1:2], b_odd[:, T:T + 1], qtop(L - 1))
    nc.vector.tensor_add(z1[:, 2:3], z1[:, 0:1], z1[:, 1:2])
    nc.scalar.activation(z1[:, 3:4], z1[:, 2:3], AF.Ln, scale=float(2.0 ** 64))
    lf = fpool.tile([BQ, 1], FP32, tag="lout")
    nc.scalar.activation(lf[:], z1[:, 3:4], AF.Identity, bias=tlogk_ap, scale=-1.0)
    nc.sync.dma_start(loss_out[:, :], lf[:])


_CACHE = {}

# The walrus build in this container accepts at most ONE sem-wait condition
# per instruction. Tile emits merged multi-waits; split the extras onto
# injected standalone EventSemaphore wait instructions on the same engine.
_WAIT_TMPL = {"debug": 0, "engine": "DVE", "ins": [], "name": "W-0",
              "opcode": "EventSemaphore", "outs": [],
              "sync_info": {"on_update": [], "on_wait": []}}


def _split_multiwaits(js: bytes) -> bytes:
    import copy
    import json
    m = json.loads(js)
    ctr = 0
    for f in m["functions"]:
        for bb in f["blocks"]:
            if "instructions" not in bb:
                continue
            out = []
            for ins in bb["instructions"]:
                si = ins.get("sync_info")
                ow = (si or {}).get("on_wait") or []
                if len(ow) > 1:
                    for wcond in ow[:-1]:
                        nop = copy.deepcopy(_WAIT_TMPL)
                        nop["engine"] = ins["engine"]
                        nop["name"] = f"W-{ctr}"
                        ctr += 1
                        nop["sync_info"]["on_wait"] = [wcond]
                        out.append(nop)
                    si["on_wait"] = [ow[-1]]
                out.append(ins)
            bb["instructions"] = out
    return json.dumps(m).encode()


def _build_nc():
    if "nc" in _CACHE:
        return _CACHE["nc"]
    nc = bass.Bass("TRN2", target_bir_lowering=False, debug=False)
    yp = nc.dram_tensor("yp", [BQ, T, C], FP32, kind="ExternalInput").ap()
    hmat = nc.dram_tensor("hmat", [C, BQ * L], BF16, kind="ExternalInput").ap()
    mask = nc.dram_tensor("mask", [BQ, L], FP32, kind="ExternalInput").ap()
    loss = nc.dram_tensor("loss", [BQ, 1], FP32, kind="ExternalOutput").ap()
    with tile.TileContext(nc) as tc:
        _ctc_tile_kernel(tc, [loss], [yp, hmat, mask])
    orig = type(nc).to_json_bytes
    nc.to_json_bytes = lambda: _split_multiwaits(orig(nc))
    _CACHE["nc"] = nc
    return nc


def _host_prep(yt_shard):
    H = np.zeros((C, BQ * L), np.float32)
    cols = np.arange(BQ * L)
    H[yt_shard.reshape(-1).astype(np.int64), cols] = KVAL
    m = np.zeros((BQ, L), np.float32)
    m[:, 1:] = (yt_shard[:, 1:] != yt_shard[:, :-1]).astype(np.float32)
    return np.ascontiguousarray(H.astype(ml_dtypes.bfloat16)), m


def _run(y_true, y_pred, trace=False):
    nc = _build_nc()
    yt_np = np.asarray(y_true)
    yp_np = np.asarray(y_pred, dtype=np.float32)
    in_maps = []
    for ci in range(NCORES):
        sl = slice(ci * BQ, (ci + 1) * BQ)
        H, m = _host_prep(yt_np[sl])
        in_maps.append({
            "yp": np.ascontiguousarray(yp_np[sl]),
            "hmat": H,
            "mask": m,
        })
    res = run_bass_kernel_spmd(nc, in_maps, core_ids=list(range(NCORES)),
                               trace=trace)
    loss = np.concatenate([res.results[ci]["loss"] for ci in range(NCORES)],
                          axis=0).astype(np.float32)
    return loss, res


def kernel(y_true, y_pred):
    loss, _ = _run(y_true, y_pred, trace=False)
    return loss


# revision 15
# speedup vs baseline: 1.3046x; 1.0043x over previous
"""CTC loss (keras ctc_batch_cost semantics, full-width lengths, blank=C-1)
as a Bass/Tile kernel on 8 TRN2 NeuronCores. Pure data parallel over batch.

Per core (128 batch rows on SBUF partitions):
  - stream y_pred shard, ACT permuted-downcast to bf16 [t,(c,b)] panels,
    xbar-transpose (8 batch rows per transpose) to c-major [c,(b,t)]
  - PE one-hot gather (entries = K) -> per-label probability slabs Q_k[b,t]
  - B-form s-sweep over the extended CTC lattice: each lattice row's whole
    time trajectory is one tensor_tensor_scan (state = coef[t]*state + inflow),
    all slab streams bf16 (DVE 2x-capable layout), scan state fp32
  - K = 78.5 per-step normalizer keeps trajectories inside fp32/bf16 range
  - loss = T*log(K) - log(2^64*(Bfin[T]*Rs[T-1] + Bodd[T]*Qs_{L-1}[T-1])) + 64*ln2
"""

import math
import sys
from contextlib import ExitStack

import numpy as np

sys.path.insert(0, "/opt/trn_rl_repo")

import ml_dtypes  # noqa: E402

import concourse.bass as bass  # noqa: E402
import concourse.tile as tile  # noqa: E402
from concourse import mybir  # noqa: E402
from concourse._compat import with_exitstack  # noqa: E402
from concourse.bass_utils import run_bass_kernel_spmd  # noqa: E402

# problem constants (hardcoded; harness shapes are fixed)
B_FULL = 1024
T = 512
C = 128
L = 64
NCORES = 8
BQ = B_FULL // NCORES   # 128 batch rows per core
S_PAD = T + 2           # slab block: col0 = pad (t=-1), cols 1..T = t, col T+1 dead
KVAL = 78.5             # bf16-exact per-step normalizer (calibrated offline)
EPS = 1e-7
EPSK = EPS * KVAL
LNSHIFT = 64.0 * math.log(2.0)   # Ln(z*2^64) keeps ACT Ln in its accurate domain
TLOGK = float(T * math.log(KVAL)) + LNSHIFT

FP32 = mybir.dt.float32
BF16 = mybir.dt.bfloat16
AF = mybir.ActivationFunctionType
ALU = mybir.AluOpType

TQ = 256                # t-panel width for the gather pipeline
NPANEL = T // TQ
BG = 16                 # batch elems per staging tile / batched transpose


@with_exitstack
def _ctc_tile_kernel(ctx: ExitStack, tc: tile.TileContext, outs, ins):
    nc = tc.nc
    yp, hmat, mask = ins
    (loss_out,) = outs

    consts = ctx.enter_context(tc.tile_pool(name="consts", bufs=1))
    bfpool = ctx.enter_context(tc.tile_pool(name="bfc", bufs=1))
    ytpool = ctx.enter_context(tc.tile_pool(name="yt", bufs=1))
    qpool = ctx.enter_context(tc.tile_pool(name="qs", bufs=1))
    psum = ctx.enter_context(tc.tile_pool(name="ps", bufs=4, space="PSUM"))
    stpool = ctx.enter_context(tc.tile_pool(name="stg", bufs=12))
    bpool = ctx.enter_context(tc.tile_pool(name="bslab", bufs=1))
    wpool = ctx.enter_context(tc.tile_pool(name="wrk", bufs=3))
    fpool = ctx.enter_context(tc.tile_pool(name="fin", bufs=1))

    hm = consts.tile([C, BQ * L], BF16, tag="hm")        # one-hot*K, [c, (b,k)]
    nc.sync.dma_start(hm[:], hmat[:, :])
    msk = consts.tile([BQ, L], FP32, tag="msk")
    nc.sync.dma_start(msk[:], mask[:, :])
    qs = qpool.tile([BQ, L * S_PAD], BF16, tag="qs")     # Q slabs, per-k blocks
    rs = consts.tile([BQ, S_PAD], BF16, tag="rs")        # blank slab
    qs3 = qs[:].rearrange("b (k s) -> b k s", k=L)
    nc.vector.memset(qs3[:, :, 0:1], 0.0)
    nc.vector.memset(rs[:, 0:1], 1.0)

    yt = ytpool.tile([C, BQ * TQ], BF16, tag="yt")       # c-major Y panel [c,(b,t)]

    cbias = consts.tile([BQ, 2], FP32, tag="cbias")
    nc.vector.memset(cbias[:, 0:1], EPSK)
    nc.vector.memset(cbias[:, 1:2], TLOGK)
    epsk_ap = cbias[:, 0:1]
    tlogk_ap = cbias[:, 1:2]

    # ---- stage A: SWDGE cast-stream + transpose + PE gather ----
    # one big casting load per 128-t chunk, then a clustered transpose burst
    # (minimizes xbar-mode transitions, which serialize the DMA path)
    for p in range(NPANEL):
        t0 = p * TQ
        for th in range(TQ // 128):
            tt = t0 + th * 128
            bfall = bfpool.tile([128, BQ * C], BF16, tag="bfall")
            for hb in range(2):
                src_ = yp[hb * 64:(hb + 1) * 64, tt:tt + 128, :]
                dst_ = bfall[:, hb * 64 * C:(hb + 1) * 64 * C]
                nc.gpsimd.dma_start(dst_.rearrange("t (b c) -> t b c", b=64),
                                    src_.rearrange("b t c -> t b c"))
            for bg in range(BQ // BG):
                dst = yt[:].rearrange("c (b t) -> c b t", b=BQ)[
                    :, bg * BG:(bg + 1) * BG, th * 128:th * 128 + 128]
                nc.sync.dma_start_transpose(
                    dst, bfall[:, bg * BG * C:(bg + 1) * BG * C])
        # blank slab: yt partition C-1 holds r[b, t] for this panel
        rraw = stpool.tile([BQ, TQ], BF16, tag="rraw")
        nc.sync.dma_start(rraw[:], yt[C - 1:C, :])
        nc.scalar.activation(rs[:, 1 + t0: 1 + t0 + TQ], rraw[:], AF.Identity,
                             bias=epsk_ap, scale=KVAL)
        # PE gather per b-pair -> PSUM [128(2b), TQ], one rearrange DMA per pair
        for bp in range(BQ // 2):
            b0, b1 = 2 * bp, 2 * bp + 1
            pt = psum.tile([128, TQ], FP32, tag="ps")
            nc.tensor.matmul(pt[0:64, :], hm[:, b0 * L:(b0 + 1) * L],
                             yt[:, b0 * TQ:(b0 + 1) * TQ], start=True, stop=True)
            nc.tensor.matmul(pt[64:128, :], hm[:, b1 * L:(b1 + 1) * L],
                             yt[:, b1 * TQ:(b1 + 1) * TQ], start=True, stop=True,
                             tile_position=(0, 64))
            st = stpool.tile([128, TQ], BF16, tag="stg")
            nc.scalar.activation(st[:], pt[:], AF.Identity, bias=epsk_ap, scale=1.0)
            dst = qs[b0:b0 + 2, :].rearrange("b (k s) -> b k s", k=L)
            nc.sync.dma_start(dst[:, :, 1 + t0:1 + t0 + TQ], st[:, :])

    # ---- stage B: the s-sweep (all-bf16 streams, fp32 scan state) ----
    rsh = rs[:, 0:T]              # shifted (t-1) view, 4B-aligned
    rtop = rs[:, T:T + 1]

    def qsh(k):
        return qs[:, k * S_PAD: k * S_PAD + T]

    def qtop(k):
        return qs[:, k * S_PAD + T: k * S_PAD + T + 1]

    # B-slab ring: pads preset once
    b0s = bpool.tile([BQ, S_PAD], BF16, tag="b0")
    ring = []
    for i in range(3):
        rtile = bpool.tile([BQ, S_PAD], BF16, tag=f"br{i}")
        ring.append(rtile)
    nc.vector.memset(b0s[:, 0:1], 1.0)
    for r_ in ring:
        nc.vector.memset(r_[:, 0:1], 0.0)

    # row 0: B0 = TTS(Rsh, e0), e0 = [1,0,...]
    w = wpool.tile([BQ, T], BF16, tag="wrk")
    nc.vector.memset(w[:], 0.0)
    nc.vector.memset(w[:, 0:1], 1.0)
    b_even = b0s
    nc.vector.tensor_tensor_scan(b_even[:, 1:T + 1], rsh, w[:], 0.0,
                                 ALU.mult, ALU.add)

    # row 1 (k=0): u = Rsh*B0sh (u[0]=1); B1 = TTS(Q0sh, u)
    u = wpool.tile([BQ, T], BF16, tag="wrk")
    nc.vector.tensor_mul(u[:], rsh, b_even[:, 0:T])
    nc.vector.memset(u[:, 0:1], 1.0)
    b_odd = ring[0]
    nc.vector.tensor_tensor_scan(b_odd[:, 1:T + 1], qsh(0), u[:], 0.0,
                                 ALU.mult, ALU.add)

    for k in range(1, L):
        tmp = wpool.tile([BQ, T], BF16, tag="wrk")
        nc.vector.tensor_mul(tmp[:], qsh(k - 1), b_odd[:, 0:T])
        b_even = ring[(2 * k) % 3]
        nc.vector.tensor_tensor_scan(b_even[:, 1:T + 1], rsh, tmp[:], 0.0,
                                     ALU.mult, ALU.add)
        u = wpool.tile([BQ, T], BF16, tag="wrk")
        nc.vector.tensor_mul(u[:], rsh, b_even[:, 0:T])
        w = wpool.tile([BQ, T], BF16, tag="wrk")
        nc.vector.scalar_tensor_tensor(w[:], tmp[:], msk[:, k:k + 1], u[:],
                                       ALU.mult, ALU.add)
        b_odd = ring[(2 * k + 1) % 3]
        nc.vector.tensor_tensor_scan(b_odd[:, 1:T + 1], qsh(k), w[:], 0.0,
                                     ALU.mult, ALU.add)

    tmp = wpool.tile([BQ, T], BF16, tag="wrk")
    nc.vector.tensor_mul(tmp[:], qsh(L - 1), b_odd[:, 0:T])
    b_fin = ring[(2 * L) % 3]
    nc.vector.tensor_tensor_scan(b_fin[:, 1:T + 1], rsh, tmp[:], 0.0,
                                 ALU.mult, ALU.add)

    # ---- final extraction ----
    z1 = fpool.tile([BQ, 4], FP32, tag="fin")
    nc.vector.tensor_mul(z1[:, 0:1], b_fin[:, T:T + 1], rtop)
    nc.vector.tensor_mul(z1[:, 1:2], b_odd[:, T:T + 1], qtop(L - 1))
    nc.vector.tensor_add(z1[:, 2:3], z1[:, 0:1], z1[:, 1:2])
    nc.scalar.activation(z1[:, 3:4], z1[:, 2:3], AF.Ln, scale=float(2.0 ** 64))
    lf = fpool.tile([BQ, 1], FP32, tag="lout")
    nc.scalar.activation(lf[:], z1[:, 3:4], AF.Identity, bias=tlogk_ap, scale=-1.0)
    nc.sync.dma_start(loss_out[:, :], lf[:])


_CACHE = {}

# The walrus build in this container accepts at most ONE sem-wait condition
# per instruction. Tile emits merged multi-waits; split the extras onto
# injected standalone EventSemaphore wait instructions on the same engine.
_WAIT_TMPL = {"debug": 0, "engine": "DVE", "ins": [], "name": "W-0",
              "opcode": "EventSemaphore", "outs": [],
              "sync_info": {"on_update": [], "on_wait": []}}


def _split_multiwaits(js: bytes) -> bytes:
    import copy
    import json
    m = json.loads(js)
    ctr = 0
    for f in m["functions"]:
        for bb in f["blocks"]:
            if "instructions" not in bb:
                continue
            out = []
            for ins in bb["instructions"]:
                si = ins.get("sync_info")
                ow = (si or {}).get("on_wait") or []
                if len(ow) > 1:
                    for wcond in ow[:-1]:
                        nop = copy.deepcopy(_WAIT_TMPL)
                        nop["engine"] = ins["engine"]
                        nop["name"] = f"W-{ctr}"
                        ctr += 1
                        nop["sync_info"]["on_wait"] = [wcond]
                        out.append(nop)
                    si["on_wait"] = [ow[-1]]
                out.append(ins)
            bb["instructions"] = out
    return json.dumps(m).encode()


def _build_nc():
    if "nc" in _CACHE:
        return _CACHE["nc"]
    nc = bass.Bass("TRN2", target_bir_lowering=False, debug=False)
    yp = nc.dram_tensor("yp", [BQ, T, C], FP32, kind="ExternalInput").ap()
    hmat = nc.dram_tensor("hmat", [C, BQ * L], BF16, kind="ExternalInput").ap()
    mask = nc.dram_tensor("mask", [BQ, L], FP32, kind="ExternalInput").ap()
    loss = nc.dram_tensor("loss", [BQ, 1], FP32, kind="ExternalOutput").ap()
    with tile.TileContext(nc) as tc:
        _ctc_tile_kernel(tc, [loss], [yp, hmat, mask])
    orig = type(nc).to_json_bytes
    nc.to_json_bytes = lambda: _split_multiwaits(orig(nc))
    _CACHE["nc"] = nc
    return nc


def _host_prep(yt_shard):
    H = np.zeros((C, BQ * L), np.float32)
    cols = np.arange(BQ * L)
    H[yt_shard.reshape(-1).astype(np.int64), cols] = KVAL
    m = np.zeros((BQ, L), np.float32)
    m[:, 1:] = (yt_shard[:, 1:] != yt_shard[:, :-1]).astype(np.float32)
    return np.ascontiguousarray(H.astype(ml_dtypes.bfloat16)), m


def _run(y_true, y_pred, trace=False):
    nc = _build_nc()
    yt_np = np.asarray(y_true)
    yp_np = np.asarray(y_pred, dtype=np.float32)
    in_maps = []
    for ci in range(NCORES):
        sl = slice(ci * BQ, (ci + 1) * BQ)
        H, m = _host_prep(yt_np[sl])
        in_maps.append({
            "yp": np.ascontiguousarray(yp_np[sl]),
            "hmat": H,
            "mask": m,
        })
    res = run_bass_kernel_spmd(nc, in_maps, core_ids=list(range(NCORES)),
                               trace=trace)
    loss = np.concatenate([res.results[ci]["loss"] for ci in range(NCORES)],
                          axis=0).astype(np.float32)
    return loss, res


def kernel(y_true, y_pred):
    loss, _ = _run(y_true, y_pred, trace=False)
    return loss


# revision 16
# speedup vs baseline: 1.4673x; 1.1247x over previous
"""CTC loss (keras ctc_batch_cost semantics, full-width lengths, blank=C-1)
as a Bass/Tile kernel on 8 TRN2 NeuronCores. Pure data parallel over batch.

Per core (128 batch rows on SBUF partitions):
  - stream y_pred shard, ACT permuted-downcast to bf16 [t,(c,b)] panels,
    xbar-transpose (8 batch rows per transpose) to c-major [c,(b,t)]
  - PE one-hot gather (entries = K) -> per-label probability slabs Q_k[b,t]
  - B-form s-sweep over the extended CTC lattice: each lattice row's whole
    time trajectory is one tensor_tensor_scan (state = coef[t]*state + inflow),
    all slab streams bf16 (DVE 2x-capable layout), scan state fp32
  - K = 78.5 per-step normalizer keeps trajectories inside fp32/bf16 range
  - loss = T*log(K) - log(2^64*(Bfin[T]*Rs[T-1] + Bodd[T]*Qs_{L-1}[T-1])) + 64*ln2
"""

import math
import sys
from contextlib import ExitStack

import numpy as np

sys.path.insert(0, "/opt/trn_rl_repo")

import ml_dtypes  # noqa: E402

import concourse.bass as bass  # noqa: E402
import concourse.tile as tile  # noqa: E402
from concourse import mybir  # noqa: E402
from concourse._compat import with_exitstack  # noqa: E402
from concourse.bass_utils import run_bass_kernel_spmd  # noqa: E402

# problem constants (hardcoded; harness shapes are fixed)
B_FULL = 1024
T = 512
C = 128
L = 64
NCORES = 8
BQ = B_FULL // NCORES   # 128 batch rows per core
S_PAD = T + 2           # slab block: col0 = pad (t=-1), cols 1..T = t, col T+1 dead
KVAL = 78.5             # bf16-exact per-step normalizer (calibrated offline)
EPS = 1e-7
EPSK = EPS * KVAL
LNSHIFT = 64.0 * math.log(2.0)   # Ln(z*2^64) keeps ACT Ln in its accurate domain
TLOGK = float(T * math.log(KVAL)) + LNSHIFT

FP32 = mybir.dt.float32
BF16 = mybir.dt.bfloat16
AF = mybir.ActivationFunctionType
ALU = mybir.AluOpType

TQ = 256                # t-panel width for the gather pipeline
NPANEL = T // TQ
BG = 16                 # batch elems per staging tile / batched transpose


@with_exitstack
def _ctc_tile_kernel(ctx: ExitStack, tc: tile.TileContext, outs, ins):
    nc = tc.nc
    yp, hmat, mask = ins
    (loss_out,) = outs

    consts = ctx.enter_context(tc.tile_pool(name="consts", bufs=1))
    bfpool = ctx.enter_context(tc.tile_pool(name="bfc", bufs=1))
    ytpool = ctx.enter_context(tc.tile_pool(name="yt", bufs=1))
    qpool = ctx.enter_context(tc.tile_pool(name="qs", bufs=1))
    psum = ctx.enter_context(tc.tile_pool(name="ps", bufs=4, space="PSUM"))
    stpool = ctx.enter_context(tc.tile_pool(name="stg", bufs=12))
    bpool = ctx.enter_context(tc.tile_pool(name="bslab", bufs=1))
    wpool = ctx.enter_context(tc.tile_pool(name="wrk", bufs=3))
    fpool = ctx.enter_context(tc.tile_pool(name="fin", bufs=1))

    hm = consts.tile([C, BQ * L], BF16, tag="hm")        # one-hot*K, [c, (b,k)]
    nc.sync.dma_start(hm[:], hmat[:, :])
    msk = consts.tile([BQ, L], FP32, tag="msk")
    nc.sync.dma_start(msk[:], mask[:, :])
    qs = qpool.tile([BQ, L * S_PAD], BF16, tag="qs")     # Q slabs, per-k blocks
    rs = consts.tile([BQ, S_PAD], BF16, tag="rs")        # blank slab
    qs3 = qs[:].rearrange("b (k s) -> b k s", k=L)
    nc.vector.memset(qs3[:, :, 0:1], 0.0)
    nc.vector.memset(rs[:, 0:1], 1.0)

    yt = ytpool.tile([C, BQ * TQ], BF16, tag="yt")       # c-major Y panel [c,(b,t)]

    cbias = consts.tile([BQ, 2], FP32, tag="cbias")
    nc.vector.memset(cbias[:, 0:1], EPSK)
    nc.vector.memset(cbias[:, 1:2], TLOGK)
    epsk_ap = cbias[:, 0:1]
    tlogk_ap = cbias[:, 1:2]

    # ---- stage A: SWDGE cast-stream + transpose + PE gather ----
    # one big casting load per 128-t chunk, then a clustered transpose burst
    # (minimizes xbar-mode transitions, which serialize the DMA path)
    for p in range(NPANEL):
        t0 = p * TQ
        for th in range(TQ // 128):
            tt = t0 + th * 128
            bfall = bfpool.tile([128, BQ * C], BF16, tag="bfall")
            for hb in range(2):
                src_ = yp[hb * 64:(hb + 1) * 64, tt:tt + 128, :]
                dst_ = bfall[:, hb * 64 * C:(hb + 1) * 64 * C]
                nc.gpsimd.dma_start(dst_.rearrange("t (b c) -> t b c", b=64),
                                    src_.rearrange("b t c -> t b c"))
            for bg in range(BQ // BG):
                dst = yt[:].rearrange("c (b t) -> c b t", b=BQ)[
                    :, bg * BG:(bg + 1) * BG, th * 128:th * 128 + 128]
                nc.sync.dma_start_transpose(
                    dst, bfall[:, bg * BG * C:(bg + 1) * BG * C])
        # blank slab: yt partition C-1 holds r[b, t] for this panel
        rraw = stpool.tile([BQ, TQ], BF16, tag="rraw")
        nc.sync.dma_start(rraw[:], yt[C - 1:C, :])
        nc.scalar.activation(rs[:, 1 + t0: 1 + t0 + TQ], rraw[:], AF.Identity,
                             bias=epsk_ap, scale=KVAL)
        # PE gather per b-pair -> PSUM [128(2b), TQ], one rearrange DMA per pair
        for bp in range(BQ // 2):
            b0, b1 = 2 * bp, 2 * bp + 1
            pt = psum.tile([128, TQ], FP32, tag="ps")
            nc.tensor.matmul(pt[0:64, :], hm[:, b0 * L:(b0 + 1) * L],
                             yt[:, b0 * TQ:(b0 + 1) * TQ], start=True, stop=True)
            nc.tensor.matmul(pt[64:128, :], hm[:, b1 * L:(b1 + 1) * L],
                             yt[:, b1 * TQ:(b1 + 1) * TQ], start=True, stop=True,
                             tile_position=(0, 64))
            st = stpool.tile([128, TQ], BF16, tag="stg")
            nc.scalar.activation(st[:], pt[:], AF.Identity, bias=epsk_ap, scale=1.0)
            dst = qs[b0:b0 + 2, :].rearrange("b (k s) -> b k s", k=L)
            nc.sync.dma_start(dst[:, :, 1 + t0:1 + t0 + TQ], st[:, :])

    # ---- stage B: t-segmented s-sweep (all-bf16 streams, fp32 scan state) ----
    # Two 256-wide time segments. Segment 0 of every lattice row runs as soon
    # as the first gather half lands (overlaps the second gather half);
    # segment 1 chains via a saved per-row boundary column.
    HS = T // 2                   # 256
    rtop = rs[:, T:T + 1]

    def qseg(k, s):
        return qs[:, k * S_PAD + s * HS: k * S_PAD + s * HS + HS]

    def rseg(s):
        return rs[:, s * HS: s * HS + HS]

    def qtop(k):
        return qs[:, k * S_PAD + T: k * S_PAD + T + 1]

    # B-slab ring + per-row boundary column (col 256 value after segment 0)
    b0s = bpool.tile([BQ, S_PAD], BF16, tag="b0")
    ring = []
    for i in range(3):
        rtile = bpool.tile([BQ, S_PAD], BF16, tag=f"br{i}")
        ring.append(rtile)
    bnd = bpool.tile([BQ, 2 * L + 2], BF16, tag="bnd")
    nc.vector.memset(b0s[:, 0:1], 1.0)
    for r_ in ring:
        nc.vector.memset(r_[:, 0:1], 0.0)

    def slab(row):
        return b0s if row == 0 else ring[(row - 1) % 3]

    zero_h = wpool.tile([BQ, HS], BF16, tag="zeroh")
    nc.vector.memset(zero_h[:], 0.0)

    # ---- segment 0 pass (t in [0, 256)) ----
    w = wpool.tile([BQ, HS], BF16, tag="wrk")
    nc.vector.memset(w[:], 0.0)
    nc.vector.memset(w[:, 0:1], 1.0)
    nc.vector.tensor_tensor_scan(b0s[:, 1:HS + 1], rseg(0), w[:], 0.0,
                                 ALU.mult, ALU.add)
    nc.vector.tensor_copy(bnd[:, 0:1], b0s[:, HS:HS + 1])

    u = wpool.tile([BQ, HS], BF16, tag="wrk")
    nc.vector.tensor_mul(u[:], rseg(0), b0s[:, 0:HS])
    nc.vector.memset(u[:, 0:1], 1.0)
    nc.vector.tensor_tensor_scan(slab(1)[:, 1:HS + 1], qseg(0, 0), u[:], 0.0,
                                 ALU.mult, ALU.add)
    nc.vector.tensor_copy(bnd[:, 1:2], slab(1)[:, HS:HS + 1])

    for k in range(1, L):
        tmp = wpool.tile([BQ, HS], BF16, tag="wrk")
        nc.vector.tensor_mul(tmp[:], qseg(k - 1, 0), slab(2 * k - 1)[:, 0:HS])
        be = slab(2 * k)
        nc.vector.tensor_tensor_scan(be[:, 1:HS + 1], rseg(0), tmp[:], 0.0,
                                     ALU.mult, ALU.add)
        nc.vector.tensor_copy(bnd[:, 2 * k:2 * k + 1], be[:, HS:HS + 1])
        u = wpool.tile([BQ, HS], BF16, tag="wrk")
        nc.vector.tensor_mul(u[:], rseg(0), be[:, 0:HS])
        w = wpool.tile([BQ, HS], BF16, tag="wrk")
        nc.vector.scalar_tensor_tensor(w[:], tmp[:], msk[:, k:k + 1], u[:],
                                       ALU.mult, ALU.add)
        bo = slab(2 * k + 1)
        nc.vector.tensor_tensor_scan(bo[:, 1:HS + 1], qseg(k, 0), w[:], 0.0,
                                     ALU.mult, ALU.add)
        nc.vector.tensor_copy(bnd[:, 2 * k + 1:2 * k + 2], bo[:, HS:HS + 1])

    tmp = wpool.tile([BQ, HS], BF16, tag="wrk")
    nc.vector.tensor_mul(tmp[:], qseg(L - 1, 0), slab(2 * L - 1)[:, 0:HS])
    bf_ = slab(2 * L)
    nc.vector.tensor_tensor_scan(bf_[:, 1:HS + 1], rseg(0), tmp[:], 0.0,
                                 ALU.mult, ALU.add)
    nc.vector.tensor_copy(bnd[:, 2 * L:2 * L + 1], bf_[:, HS:HS + 1])

    # ---- segment 1 pass (t in [256, 512)) ----
    # restore each row's boundary col (t=255) into its ring slot, then scan on
    def restore(row):
        nc.vector.tensor_copy(slab(row)[:, HS:HS + 1], bnd[:, row:row + 1])

    restore(0)
    nc.vector.tensor_tensor_scan(b0s[:, HS + 1:T + 1], rseg(1), zero_h[:],
                                 bnd[:, 0:1], ALU.mult, ALU.add)
    restore(1)
    u = wpool.tile([BQ, HS], BF16, tag="wrk")
    nc.vector.tensor_mul(u[:], rseg(1), b0s[:, HS:T])
    nc.vector.tensor_tensor_scan(slab(1)[:, HS + 1:T + 1], qseg(0, 1), u[:],
                                 bnd[:, 1:2], ALU.mult, ALU.add)

    for k in range(1, L):
        be, bo_prev = slab(2 * k), slab(2 * k - 1)
        tmp = wpool.tile([BQ, HS], BF16, tag="wrk")
        nc.vector.tensor_mul(tmp[:], qseg(k - 1, 1), bo_prev[:, HS:T])
        restore(2 * k)
        nc.vector.tensor_tensor_scan(be[:, HS + 1:T + 1], rseg(1), tmp[:],
                                     bnd[:, 2 * k:2 * k + 1], ALU.mult, ALU.add)
        u = wpool.tile([BQ, HS], BF16, tag="wrk")
        nc.vector.tensor_mul(u[:], rseg(1), be[:, HS:T])
        w = wpool.tile([BQ, HS], BF16, tag="wrk")
        nc.vector.scalar_tensor_tensor(w[:], tmp[:], msk[:, k:k + 1], u[:],
                                       ALU.mult, ALU.add)
        restore(2 * k + 1)
        nc.vector.tensor_tensor_scan(slab(2 * k + 1)[:, HS + 1:T + 1],
                                     qseg(k, 1), w[:],
                                     bnd[:, 2 * k + 1:2 * k + 2],
                                     ALU.mult, ALU.add)

    tmp = wpool.tile([BQ, HS], BF16, tag="wrk")
    nc.vector.tensor_mul(tmp[:], qseg(L - 1, 1), slab(2 * L - 1)[:, HS:T])
    b_fin = slab(2 * L)
    nc.vector.tensor_tensor_scan(b_fin[:, HS + 1:T + 1], rseg(1), tmp[:],
                                 bnd[:, 2 * L:2 * L + 1], ALU.mult, ALU.add)
    b_odd = slab(2 * L - 1)

    # ---- final extraction ----
    z1 = fpool.tile([BQ, 4], FP32, tag="fin")
    nc.vector.tensor_mul(z1[:, 0:1], b_fin[:, T:T + 1], rtop)
    nc.vector.tensor_mul(z1[:, 1:2], b_odd[:, T:T + 1], qtop(L - 1))
    nc.vector.tensor_add(z1[:, 2:3], z1[:, 0:1], z1[:, 1:2])
    nc.scalar.activation(z1[:, 3:4], z1[:, 2:3], AF.Ln, scale=float(2.0 ** 64))
    lf = fpool.tile([BQ, 1], FP32, tag="lout")
    nc.scalar.activation(lf[:], z1[:, 3:4], AF.Identity, bias=tlogk_ap, scale=-1.0)
    nc.sync.dma_start(loss_out[:, :], lf[:])


_CACHE = {}

# The walrus build in this container accepts at most ONE sem-wait condition
# per instruction. Tile emits merged multi-waits; split the extras onto
# injected standalone EventSemaphore wait instructions on the same engine.
_WAIT_TMPL = {"debug": 0, "engine": "DVE", "ins": [], "name": "W-0",
              "opcode": "EventSemaphore", "outs": [],
              "sync_info": {"on_update": [], "on_wait": []}}


def _split_multiwaits(js: bytes) -> bytes:
    import copy
    import json
    m = json.loads(js)
    ctr = 0
    for f in m["functions"]:
        for bb in f["blocks"]:
            if "instructions" not in bb:
                continue
            out = []
            for ins in bb["instructions"]:
                si = ins.get("sync_info")
                ow = (si or {}).get("on_wait") or []
                if len(ow) > 1:
                    for wcond in ow[:-1]:
                        nop = copy.deepcopy(_WAIT_TMPL)
                        nop["engine"] = ins["engine"]
                        nop["name"] = f"W-{ctr}"
                        ctr += 1
                        nop["sync_info"]["on_wait"] = [wcond]
                        out.append(nop)
                    si["on_wait"] = [ow[-1]]
                out.append(ins)
            bb["instructions"] = out
    return json.dumps(m).encode()


def _build_nc():
    if "nc" in _CACHE:
        return _CACHE["nc"]
    nc = bass.Bass("TRN2", target_bir_lowering=False, debug=False)
    yp = nc.dram_tensor("yp", [BQ, T, C], FP32, kind="ExternalInput").ap()
    hmat = nc.dram_tensor("hmat", [C, BQ * L], BF16, kind="ExternalInput").ap()
    mask = nc.dram_tensor("mask", [BQ, L], FP32, kind="ExternalInput").ap()
    loss = nc.dram_tensor("loss", [BQ, 1], FP32, kind="ExternalOutput").ap()
    with tile.TileContext(nc) as tc:
        _ctc_tile_kernel(tc, [loss], [yp, hmat, mask])
    orig = type(nc).to_json_bytes
    nc.to_json_bytes = lambda: _split_multiwaits(orig(nc))
    _CACHE["nc"] = nc
    return nc


def _host_prep(yt_shard):
    H = np.zeros((C, BQ * L), np.float32)
    cols = np.arange(BQ * L)
    H[yt_shard.reshape(-1).astype(np.int64), cols] = KVAL
    m = np.zeros((BQ, L), np.float32)
    m[:, 1:] = (yt_shard[:, 1:] != yt_shard[:, :-1]).astype(np.float32)
    return np.ascontiguousarray(H.astype(ml_dtypes.bfloat16)), m


def _run(y_true, y_pred, trace=False):
    nc = _build_nc()
    yt_np = np.asarray(y_true)
    yp_np = np.asarray(y_pred, dtype=np.float32)
    in_maps = []
    for ci in range(NCORES):
        sl = slice(ci * BQ, (ci + 1) * BQ)
        H, m = _host_prep(yt_np[sl])
        in_maps.append({
            "yp": np.ascontiguousarray(yp_np[sl]),
            "hmat": H,
            "mask": m,
        })
    res = run_bass_kernel_spmd(nc, in_maps, core_ids=list(range(NCORES)),
                               trace=trace)
    loss = np.concatenate([res.results[ci]["loss"] for ci in range(NCORES)],
                          axis=0).astype(np.float32)
    return loss, res


def kernel(y_true, y_pred):
    loss, _ = _run(y_true, y_pred, trace=False)
    return loss
